# revision 7
# baseline (speedup 1.0000x reference)
"""Trainium2 Bass kernel for EnhancedMultiHeadAttention (B=2, S=2048, DM=1024, H=16).

Sharding: 8 cores = 2 batches x 4 query-row blocks of 512. Each core computes
K/V for its whole batch, attention + output projection + gate + layernorm for
its 512 query rows. No collectives.

v3: single fused pipeline. Q^T is projected up front; K rows and V column
quarters are projected just-in-time per head pair, SBUF-resident (no DRAM
staging). The out-projection (both orientations) is accumulated per pair in
SBUF as each pair's context completes. All of this PE work fills the gaps
under the ScalarE GELU stream (the serial ~148us bottleneck) and keeps the
PE clock un-throttled. Only the gate matmul + epilogue trail the window.

Matmuls run in fp32r (full PE speed, ~1.5e-4 per-matmul error); the
attn @ v step runs in bf16 so the two heads of a pair can be col-packed
into one PSUM bank (fp32r cannot target PSUM partitions 64-127).
"""
import math
import os
import sys

import numpy as np

for _p in ("/opt/trn_rl_repo", "/opt/pypackages"):
    if _p not in sys.path:
        sys.path.append(_p)

import concourse.bass as bass
import concourse.mybir as mybir
import concourse.tile as tile
from concourse import bacc
from concourse.bass_utils import run_bass_kernel_spmd

F32R = mybir.dt.float32r
F32 = mybir.dt.float32
BF16 = mybir.dt.bfloat16
AF = mybir.ActivationFunctionType
ALU = mybir.AluOpType

B, S, DM, H = 2, 2048, 1024, 16
HD = DM // H                  # 64
SQ = 512                      # query rows per core
NP = 128                      # partitions
KC = DM // NP                 # 8 contraction chunks
NT = S // NP                  # 16 key/value tiles
NPAIR = H // 2                # 8 head pairs
NST = SQ // NP                # 4 row tiles in row-layout phases
N512 = 512
NQ = 256                      # v-projection column quarter width
SCALE = 1.0 / math.sqrt(HD)
EPS = 1e-5

_CACHE = {}
_TRACE = [False]
_LAST_RESULT = [None]


def _bcast(ap_1d, p=NP):
    return bass.AP(tensor=ap_1d.tensor, offset=ap_1d.offset,
                   ap=[[0, p]] + list(ap_1d.ap))


def _build():
    nc = bacc.Bacc("TRN2", target_bir_lowering=False, debug=False)

    xT_d = nc.dram_tensor("xT", [DM, S], F32R, kind="ExternalInput").ap()
    xr_d = nc.dram_tensor("xr", [SQ, DM], F32, kind="ExternalInput").ap()
    wkT_d = nc.dram_tensor("wkT", [DM, DM], F32R, kind="ExternalInput").ap()
    wvT_d = nc.dram_tensor("wvT", [DM, DM], F32R, kind="ExternalInput").ap()
    wqT_d = nc.dram_tensor("wqT", [DM, DM], F32R, kind="ExternalInput").ap()
    woT_d = nc.dram_tensor("woT", [DM, DM], F32R, kind="ExternalInput").ap()
    wgT_d = nc.dram_tensor("wgT", [DM, DM], F32R, kind="ExternalInput").ap()
    bq_d = nc.dram_tensor("bq", [DM], F32, kind="ExternalInput").ap()
    bk_d = nc.dram_tensor("bk", [DM], F32, kind="ExternalInput").ap()
    bv_d = nc.dram_tensor("bv", [DM], F32, kind="ExternalInput").ap()
    bo_d = nc.dram_tensor("bo", [DM], F32, kind="ExternalInput").ap()
    bg_d = nc.dram_tensor("bg", [DM], F32, kind="ExternalInput").ap()
    gam_d = nc.dram_tensor("gam", [DM], F32, kind="ExternalInput").ap()
    bet_d = nc.dram_tensor("bet", [DM], F32, kind="ExternalInput").ap()
    y_d = nc.dram_tensor("y", [SQ, DM], F32, kind="ExternalOutput").ap()

    xT_v = xT_d.rearrange("(c p) s -> p c s", p=NP)
    wk_v = wkT_d.rearrange("(c p) d -> p c d", p=NP)
    wv_v = wvT_d.rearrange("(c p) d -> p c d", p=NP)
    wq_v = wqT_d.rearrange("(c p) d -> p c d", p=NP)
    wo_v = woT_d.rearrange("(c p) d -> p c d", p=NP)
    wg_v = wgT_d.rearrange("(c p) d -> p c d", p=NP)

    with tile.TileContext(nc) as tc:
        with tc.tile_pool(name="pers", bufs=1) as pers, \
             tc.tile_pool(name="acc", bufs=1) as acc:
            bq_sb = pers.tile([NP, KC], F32)
            bk_sb = pers.tile([NP, KC], F32)
            bo_sb = pers.tile([NP, KC], F32)
            nc.sync.dma_start(out=bq_sb, in_=bq_d.rearrange("(c p) -> p c", p=NP))
            nc.sync.dma_start(out=bk_sb, in_=bk_d.rearrange("(c p) -> p c", p=NP))
            nc.sync.dma_start(out=bo_sb, in_=bo_d.rearrange("(c p) -> p c", p=NP))
            bv_bc = pers.tile([NP, DM], F32)
            nc.sync.dma_start(out=bv_bc, in_=_bcast(bv_d))
            eps_sb = pers.tile([NP, 1], F32)
            nc.vector.memset(eps_sb, EPS)

            # accumulated across the whole window
            outT_sb = acc.tile([NP, KC, SQ], F32R)
            orow_sb = acc.tile([NP, NST, DM], F32)
            bo_bc = pers.tile([NP, DM], F32)
            nc.sync.dma_start(out=bo_bc, in_=_bcast(bo_d))

            with tc.tile_pool(name="xres", bufs=1) as xres, \
                 tc.tile_pool(name="qres", bufs=1) as qres, \
                 tc.tile_pool(name="wvp", bufs=1) as wvp, \
                 tc.tile_pool(name="wsl", bufs=2) as wsl, \
                 tc.tile_pool(name="wosl", bufs=1) as wosl, \
                 tc.tile_pool(name="kpp", bufs=2) as kpp, \
                 tc.tile_pool(name="vqp", bufs=2) as vqp, \
                 tc.tile_pool(name="attp", bufs=3) as attp, \
                 tc.tile_pool(name="ctxp", bufs=2) as ctxp, \
                 tc.tile_pool(name="pp", bufs=2, space="PSUM") as pp, \
                 tc.tile_pool(name="scop", bufs=2, space="PSUM") as scop, \
                 tc.tile_pool(name="cxp", bufs=2, space="PSUM") as cxp:
                xT_sb = xres.tile([NP, KC, S], F32R)
                for kc in range(KC):
                    nc.sync.dma_start(out=xT_sb[:, kc, :], in_=xT_v[:, kc, :])
                qT_sb = qres.tile([NP, KC, SQ], F32R)

                # ---- Q^T projection up front; query columns are the
                # first SQ columns of xT (host permutes the t order) ----
                for dt in range(KC):
                    wq_sl = wsl.tile([NP, KC, NP], F32R, tag="wq", name="wq_sl")
                    nc.sync.dma_start(out=wq_sl,
                                      in_=wq_v[:, :, dt * NP:(dt + 1) * NP])
                    ps_q = pp.tile([NP, SQ], F32, tag="pj", name="ps_q")
                    for kc in range(KC):
                        nc.tensor.matmul(ps_q, wq_sl[:, kc, :],
                                         xT_sb[:, kc, 0:SQ],
                                         start=(kc == 0), stop=(kc == KC - 1))
                    nc.vector.tensor_scalar_add(qT_sb[:, dt, :], ps_q,
                                                bq_sb[:, dt:dt + 1])

                v_q = [None] * 4

                def v_quarter(q):
                    wv_sb = wvp.tile([NP, KC, NQ], F32R, tag="wv", name="wv_sb")
                    nc.sync.dma_start(out=wv_sb,
                                      in_=wv_v[:, :, q * NQ:(q + 1) * NQ])
                    vq = vqp.tile([NP, NT, NQ], BF16, tag="vq", name="vq")
                    for tt in range(NT):
                        ps_t = pp.tile([NP, NQ], F32, tag="pj", name="ps_t")
                        for kc in range(KC):
                            nc.tensor.matmul(
                                ps_t,
                                xT_sb[:, kc, tt * NP:(tt + 1) * NP],
                                wv_sb[:, kc, :],
                                start=(kc == 0), stop=(kc == KC - 1))
                        nc.vector.tensor_add(
                            vq[:, tt, :], ps_t, bv_bc[:, q * NQ:(q + 1) * NQ])
                    v_q[q] = vq

                kpairs = [None] * NPAIR

                def prepare(p):
                    # kpair[d, t] = sum_k Wk[d, k] x[t, k] + bk[d], d in pair rows
                    wk_sl = wsl.tile([NP, KC, NP], F32R, tag="wk", name="wk_sl")
                    nc.sync.dma_start(out=wk_sl, in_=wk_v[:, :, p * NP:(p + 1) * NP])
                    kpair = kpp.tile([NP, S], F32R, tag="kp", name="kpair")
                    for ts in range(S // N512):
                        ps_t = pp.tile([NP, N512], F32, tag="pj", name="ps_t")
                        for kc in range(KC):
                            nc.tensor.matmul(
                                ps_t,
                                wk_sl[:, kc, :],
                                xT_sb[:, kc, ts * N512:(ts + 1) * N512],
                                start=(kc == 0), stop=(kc == KC - 1))
                        nc.vector.tensor_scalar_add(
                            kpair[:, ts * N512:(ts + 1) * N512], ps_t,
                            bk_sb[:, p:p + 1])
                    kpairs[p] = kpair

                def attn(p):
                    kpair = kpairs[p]
                    vq = v_q[p // 2]
                    c0 = (p % 2) * NP
                    ctx_ps = cxp.tile([NP, SQ], F32, tag="cx", name="ctx_ps")
                    for t in range(NT):
                        sco = scop.tile([NP, 2 * SQ], F32, tag="sc", name="sco")
                        nc.tensor.matmul(sco[:, 0:SQ],
                                         kpair[0:64, t * NP:(t + 1) * NP],
                                         qT_sb[0:64, p, :],
                                         start=True, stop=True,
                                         tile_position=(0, 0))
                        nc.tensor.matmul(sco[:, SQ:2 * SQ],
                                         kpair[64:128, t * NP:(t + 1) * NP],
                                         qT_sb[64:128, p, :],
                                         start=True, stop=True,
                                         tile_position=(64, 0))
                        att_t = attp.tile([NP, 2 * SQ], BF16, tag="at", name="att_t")
                        nc.scalar.activation(out=att_t, in_=sco, func=AF.Gelu,
                                             scale=SCALE)
                        nc.tensor.matmul(ctx_ps[0:64, :], vq[:, t, c0:c0 + 64],
                                         att_t[:, 0:SQ],
                                         start=(t == 0), stop=(t == NT - 1),
                                         tile_position=(0, 0))
                        nc.tensor.matmul(ctx_ps[64:128, :], vq[:, t, c0 + 64:c0 + NP],
                                         att_t[:, SQ:2 * SQ],
                                         start=(t == 0), stop=(t == NT - 1),
                                         tile_position=(0, 64))
                    ctxp_sb = ctxp.tile([NP, SQ], F32R, tag="ct", name="ctxp_sb")
                    nc.vector.tensor_copy(ctxp_sb, ctx_ps)
                    # fold this pair's context into both out-projection layouts
                    wo_sl = wosl.tile([NP, DM], F32R, tag="wo", name="wo_sl")
                    nc.sync.dma_start(out=wo_sl, in_=wo_v[:, p, :])
                    for dt in range(KC):
                        ps_t = pp.tile([NP, SQ], F32, tag="pj", name="ps_t")
                        nc.tensor.matmul(ps_t, wo_sl[:, dt * NP:(dt + 1) * NP],
                                         ctxp_sb, start=True, stop=True)
                        if p == 0:
                            nc.vector.tensor_scalar_add(
                                outT_sb[:, dt, :], ps_t, bo_sb[:, dt:dt + 1])
                        else:
                            nc.vector.tensor_add(
                                outT_sb[:, dt, :], outT_sb[:, dt, :], ps_t)
                    for st in range(NST):
                        for ns in range(DM // N512):
                            ps_t = pp.tile([NP, N512], F32, tag="pj", name="ps_t")
                            nc.tensor.matmul(
                                ps_t, ctxp_sb[:, st * NP:(st + 1) * NP],
                                wo_sl[:, ns * N512:(ns + 1) * N512],
                                start=True, stop=True)
                            sl = orow_sb[:, st, ns * N512:(ns + 1) * N512]
                            if p == 0:
                                nc.vector.tensor_add(
                                    sl, ps_t, bo_bc[:, ns * N512:(ns + 1) * N512])
                            else:
                                nc.vector.tensor_add(sl, sl, ps_t)

                prepare(0)
                v_quarter(0)
                prepare(1)
                attn(0)
                v_quarter(1)
                attn(1)
                prepare(2)
                attn(2)
                prepare(3)
                v_quarter(2)
                attn(3)
                prepare(4)
                attn(4)
                prepare(5)
                v_quarter(3)
                attn(5)
                prepare(6)
                attn(6)
                prepare(7)
                attn(7)

            # ---------------- gate + epilogue ------------------------------
            with tc.tile_pool(name="w2", bufs=1) as w2, \
                 tc.tile_pool(name="big", bufs=1) as big, \
                 tc.tile_pool(name="pp2", bufs=4, space="PSUM") as pp2:
                wg_sb = w2.tile([NP, KC, DM], F32R)
                for dt in range(KC):
                    nc.sync.dma_start(out=wg_sb[:, :, dt * NP:(dt + 1) * NP],
                                      in_=wg_v[:, :, dt * NP:(dt + 1) * NP])
                bg_bc = big.tile([NP, DM], F32)
                gam_bc = big.tile([NP, DM], F32)
                bet_bc = big.tile([NP, DM], F32)
                nc.sync.dma_start(out=bg_bc, in_=_bcast(bg_d))
                nc.sync.dma_start(out=gam_bc, in_=_bcast(gam_d))
                nc.sync.dma_start(out=bet_bc, in_=_bcast(bet_d))
                xr_sb = big.tile([NP, NST, DM], F32)
                nc.sync.dma_start(out=xr_sb,
                                  in_=xr_d.rearrange("(n p) d -> p n d", p=NP))

                gate_sb = big.tile([NP, NST, DM], F32)
                t1_sb = big.tile([NP, NST, DM], F32)
                stats = pers.tile([NP, 2, 6], F32)
                mv = pers.tile([NP, 2], F32)
                std = pers.tile([NP, 1], F32)
                rstd = pers.tile([NP, 1], F32)
                y_sb = gate_sb
                for st in range(NST):
                    for ns in range(DM // N512):
                        ps_t = pp2.tile([NP, N512], F32, tag="po", name="ps_t")
                        for dc in range(KC):
                            nc.tensor.matmul(
                                ps_t,
                                outT_sb[:, dc, st * NP:(st + 1) * NP],
                                wg_sb[:, dc, ns * N512:(ns + 1) * N512],
                                start=(dc == 0), stop=(dc == KC - 1))
                        nc.vector.tensor_add(
                            gate_sb[:, st, ns * N512:(ns + 1) * N512],
                            ps_t, bg_bc[:, ns * N512:(ns + 1) * N512])
                    nc.scalar.activation(out=gate_sb[:, st, :],
                                         in_=gate_sb[:, st, :], func=AF.Sigmoid)
                    # y_pre = gate*(out - x) + 2x
                    nc.vector.tensor_sub(t1_sb[:, st, :], orow_sb[:, st, :],
                                         xr_sb[:, st, :])
                    nc.vector.tensor_mul(orow_sb[:, st, :], t1_sb[:, st, :],
                                         gate_sb[:, st, :])
                    nc.vector.scalar_tensor_tensor(
                        out=t1_sb[:, st, :], in0=xr_sb[:, st, :], scalar=2.0,
                        in1=orow_sb[:, st, :], op0=ALU.mult, op1=ALU.add)
                    # layernorm over DM
                    yv = t1_sb[:, st, :].rearrange("p (g d) -> p g d", g=2)
                    for g in range(2):
                        nc.vector.bn_stats(out=stats[:, g, :], in_=yv[:, g, :])
                    nc.vector.bn_aggr(out=mv, in_=stats)
                    nc.scalar.activation(out=std, in_=mv[:, 1:2], func=AF.Sqrt,
                                         bias=eps_sb)
                    nc.vector.reciprocal(rstd, std)
                    nc.vector.tensor_scalar(
                        out=orow_sb[:, st, :], in0=t1_sb[:, st, :],
                        scalar1=mv[:, 0:1], scalar2=rstd,
                        op0=ALU.subtract, op1=ALU.mult)
                    nc.vector.tensor_mul(orow_sb[:, st, :], orow_sb[:, st, :],
                                         gam_bc)
                    nc.vector.tensor_add(y_sb[:, st, :], orow_sb[:, st, :],
                                         bet_bc)
                    nc.sync.dma_start(
                        out=y_d.rearrange("(n p) d -> p n d", p=NP)[:, st, :],
                        in_=y_sb[:, st, :])

    nc.compile()
    return nc


def kernel(x, Wq, bq, Wk, bk, Wv, bv, Wo, bo, Wg, bg, attention_weights,
           ln_gamma, ln_beta):
    x = np.asarray(x, dtype=np.float32)
    f32 = lambda a: np.ascontiguousarray(np.asarray(a, dtype=np.float32))
    Wq, Wk, Wv, Wo, Wg = map(f32, (Wq, Wk, Wv, Wo, Wg))
    bq, bk, bv, bo, bg = map(f32, (bq, bk, bv, bo, bg))
    aw, gam, bet = map(f32, (attention_weights, ln_gamma, ln_beta))

    if "nc" not in _CACHE:
        _CACHE["nc"] = _build()
    nc = _CACHE["nc"]

    # fold softmax(attention_weights) into Wv / bv
    e = np.exp(aw - aw.max())
    head_w = (e / e.sum()).astype(np.float32)
    hw_exp = np.repeat(head_w, HD)              # [DM]
    Wv_s = Wv * hw_exp[:, None]
    bv_s = bv * hw_exp

    wqT = np.ascontiguousarray(Wq.T)
    wkT = np.ascontiguousarray(Wk.T)
    wvT = np.ascontiguousarray(Wv_s.T)
    woT = np.ascontiguousarray(Wo.T)
    wgT = np.ascontiguousarray(Wg.T)

    in_maps = []
    for c in range(8):
        b, blk = divmod(c, 4)
        r0 = blk * SQ
        xb = x[b]
        perm = np.r_[r0:r0 + SQ, 0:r0, r0 + SQ:S]
        in_maps.append({
            "xT": np.ascontiguousarray(xb[perm].T),
            "xr": np.ascontiguousarray(xb[r0:r0 + SQ]),
            "wkT": wkT, "wvT": wvT, "wqT": wqT, "woT": woT, "wgT": wgT,
            "bq": bq, "bk": bk, "bv": bv_s, "bo": bo, "bg": bg,
            "gam": gam, "bet": bet,
        })

    res = run_bass_kernel_spmd(nc, in_maps, core_ids=list(range(8)),
                               trace=_TRACE[0])
    _LAST_RESULT[0] = res

    y = np.empty((B, S, DM), dtype=np.float32)
    for c in range(8):
        b, blk = divmod(c, 4)
        r0 = blk * SQ
        y[b, r0:r0 + SQ] = res.results[c]["y"]
    return y


# revision 8
# speedup vs baseline: 1.3157x; 1.3157x over previous
"""Trainium2 Bass kernel for EnhancedMultiHeadAttention (B=2, S=2048, DM=1024, H=16).

Sharding: 8 cores = 2 batches x 4 query-row blocks of 512. Each core computes
K/V for its whole batch, attention + output projection + gate + layernorm for
its 512 query rows. No collectives.

v3: single fused pipeline. Q^T is projected up front; K rows and V column
quarters are projected just-in-time per head pair, SBUF-resident (no DRAM
staging). The out-projection (both orientations) is accumulated per pair in
SBUF as each pair's context completes. All of this PE work fills the gaps
under the ScalarE GELU stream (the serial ~148us bottleneck) and keeps the
PE clock un-throttled. Only the gate matmul + epilogue trail the window.

Matmuls run in fp32r (full PE speed, ~1.5e-4 per-matmul error); the
attn @ v step runs in bf16 so the two heads of a pair can be col-packed
into one PSUM bank (fp32r cannot target PSUM partitions 64-127).
"""
import math
import os
import sys

import numpy as np

for _p in ("/opt/trn_rl_repo", "/opt/pypackages"):
    if _p not in sys.path:
        sys.path.append(_p)

import concourse.bass as bass
import concourse.mybir as mybir
import concourse.tile as tile
from concourse import bacc
from concourse.bass_utils import run_bass_kernel_spmd

F32R = mybir.dt.float32r
F32 = mybir.dt.float32
BF16 = mybir.dt.bfloat16
AF = mybir.ActivationFunctionType
ALU = mybir.AluOpType

B, S, DM, H = 2, 2048, 1024, 16
HD = DM // H                  # 64
SQ = 512                      # query rows per core
NP = 128                      # partitions
KC = DM // NP                 # 8 contraction chunks
NT = S // NP                  # 16 key/value tiles
NPAIR = H // 2                # 8 head pairs
NST = SQ // NP                # 4 row tiles in row-layout phases
N512 = 512
NQ = 512                      # v-projection column half width
SCALE = 1.0 / math.sqrt(HD)
EPS = 1e-5

_CACHE = {}
_TRACE = [False]
_LAST_RESULT = [None]


def _bcast(ap_1d, p=NP):
    return bass.AP(tensor=ap_1d.tensor, offset=ap_1d.offset,
                   ap=[[0, p]] + list(ap_1d.ap))


def _build():
    nc = bacc.Bacc("TRN2", target_bir_lowering=False, debug=False)

    xT_d = nc.dram_tensor("xT", [DM, S], F32R, kind="ExternalInput").ap()
    xr_d = nc.dram_tensor("xr", [SQ, DM], F32, kind="ExternalInput").ap()
    wkT_d = nc.dram_tensor("wkT", [DM, DM], F32R, kind="ExternalInput").ap()
    wvT_d = nc.dram_tensor("wvT", [DM, DM], F32R, kind="ExternalInput").ap()
    wqT_d = nc.dram_tensor("wqT", [DM, DM], F32R, kind="ExternalInput").ap()
    woT_d = nc.dram_tensor("woT", [DM, DM], F32R, kind="ExternalInput").ap()
    wgT_d = nc.dram_tensor("wgT", [DM, DM], F32R, kind="ExternalInput").ap()
    bq_d = nc.dram_tensor("bq", [DM], F32, kind="ExternalInput").ap()
    bk_d = nc.dram_tensor("bk", [DM], F32, kind="ExternalInput").ap()
    bv_d = nc.dram_tensor("bv", [DM], F32, kind="ExternalInput").ap()
    bo_d = nc.dram_tensor("bo", [DM], F32, kind="ExternalInput").ap()
    bg_d = nc.dram_tensor("bg", [DM], F32, kind="ExternalInput").ap()
    gam_d = nc.dram_tensor("gam", [DM], F32, kind="ExternalInput").ap()
    bet_d = nc.dram_tensor("bet", [DM], F32, kind="ExternalInput").ap()
    y_d = nc.dram_tensor("y", [SQ, DM], F32, kind="ExternalOutput").ap()

    xT_v = xT_d.rearrange("(c p) s -> p c s", p=NP)
    wk_v = wkT_d.rearrange("(c p) d -> p c d", p=NP)
    wv_v = wvT_d.rearrange("(c p) d -> p c d", p=NP)
    wq_v = wqT_d.rearrange("(c p) d -> p c d", p=NP)
    wo_v = woT_d.rearrange("(c p) d -> p c d", p=NP)
    wg_v = wgT_d.rearrange("(c p) d -> p c d", p=NP)

    with tile.TileContext(nc) as tc:
        with tc.tile_pool(name="pers", bufs=1) as pers, \
             tc.tile_pool(name="acc", bufs=1) as acc:
            bq_sb = pers.tile([NP, KC], F32)
            bk_sb = pers.tile([NP, KC], F32)
            bo_sb = pers.tile([NP, KC], F32)
            nc.sync.dma_start(out=bq_sb, in_=bq_d.rearrange("(c p) -> p c", p=NP))
            nc.sync.dma_start(out=bk_sb, in_=bk_d.rearrange("(c p) -> p c", p=NP))
            nc.sync.dma_start(out=bo_sb, in_=bo_d.rearrange("(c p) -> p c", p=NP))
            bv_bc = pers.tile([NP, DM], F32)
            nc.sync.dma_start(out=bv_bc, in_=_bcast(bv_d))
            eps_sb = pers.tile([NP, 1], F32)
            nc.vector.memset(eps_sb, EPS)

            ctxT_sb = acc.tile([NP, NPAIR, SQ], F32R)

            with tc.tile_pool(name="xres", bufs=1) as xres, \
                 tc.tile_pool(name="qres", bufs=1) as qres, \
                 tc.tile_pool(name="wvp", bufs=1) as wvp, \
                 tc.tile_pool(name="wsl", bufs=2) as wsl, \
                 tc.tile_pool(name="kpp", bufs=2) as kpp, \
                 tc.tile_pool(name="vqp", bufs=2) as vqp, \
                 tc.tile_pool(name="attp", bufs=3) as attp, \
                 tc.tile_pool(name="pp", bufs=2, space="PSUM") as pp, \
                 tc.tile_pool(name="scop", bufs=2, space="PSUM") as scop, \
                 tc.tile_pool(name="cxp", bufs=2, space="PSUM") as cxp:
                xT_sb = xres.tile([NP, KC, S], F32R)
                for kc in range(KC):
                    nc.sync.dma_start(out=xT_sb[:, kc, :], in_=xT_v[:, kc, :])
                qT_sb = qres.tile([NP, KC, SQ], F32R)

                # ---- Q^T projection up front; query columns are the
                # first SQ columns of xT (host permutes the t order) ----
                for dt in range(KC):
                    wq_sl = wsl.tile([NP, KC, NP], F32R, tag="wq", name="wq_sl")
                    nc.sync.dma_start(out=wq_sl,
                                      in_=wq_v[:, :, dt * NP:(dt + 1) * NP])
                    ps_q = pp.tile([NP, SQ], F32, tag="pj", name="ps_q")
                    for kc in range(KC):
                        nc.tensor.matmul(ps_q, wq_sl[:, kc, :],
                                         xT_sb[:, kc, 0:SQ],
                                         start=(kc == 0), stop=(kc == KC - 1))
                    nc.vector.tensor_scalar_add(qT_sb[:, dt, :], ps_q,
                                                bq_sb[:, dt:dt + 1])

                v_q = [None] * 2

                def v_half(q):
                    wv_sb = wvp.tile([NP, KC, NQ], F32R, tag="wv", name="wv_sb")
                    nc.sync.dma_start(out=wv_sb,
                                      in_=wv_v[:, :, q * NQ:(q + 1) * NQ])
                    vq = vqp.tile([NP, NT, NQ], BF16, tag="vq", name="vq")
                    for tt in range(NT):
                        ps_t = pp.tile([NP, NQ], F32, tag="pj", name="ps_t")
                        for kc in range(KC):
                            nc.tensor.matmul(
                                ps_t,
                                xT_sb[:, kc, tt * NP:(tt + 1) * NP],
                                wv_sb[:, kc, :],
                                start=(kc == 0), stop=(kc == KC - 1))
                        nc.vector.tensor_add(
                            vq[:, tt, :], ps_t, bv_bc[:, q * NQ:(q + 1) * NQ])
                    v_q[q] = vq

                kpairs = [None] * NPAIR

                def prepare(p):
                    # kpair[d, t] = sum_k Wk[d, k] x[t, k] + bk[d], d in pair rows
                    wk_sl = wsl.tile([NP, KC, NP], F32R, tag="wk", name="wk_sl")
                    nc.sync.dma_start(out=wk_sl, in_=wk_v[:, :, p * NP:(p + 1) * NP])
                    kpair = kpp.tile([NP, S], F32R, tag="kp", name="kpair")
                    for ts in range(S // N512):
                        ps_t = pp.tile([NP, N512], F32, tag="pj", name="ps_t")
                        for kc in range(KC):
                            nc.tensor.matmul(
                                ps_t,
                                wk_sl[:, kc, :],
                                xT_sb[:, kc, ts * N512:(ts + 1) * N512],
                                start=(kc == 0), stop=(kc == KC - 1))
                        nc.vector.tensor_scalar_add(
                            kpair[:, ts * N512:(ts + 1) * N512], ps_t,
                            bk_sb[:, p:p + 1])
                    kpairs[p] = kpair

                def attn(p):
                    kpair = kpairs[p]
                    vq = v_q[p // 4]
                    c0 = (p % 4) * NP
                    ctx_ps = cxp.tile([NP, SQ], F32, tag="cx", name="ctx_ps")
                    for t in range(NT):
                        sco = scop.tile([NP, 2 * SQ], F32, tag="sc", name="sco")
                        nc.tensor.matmul(sco[:, 0:SQ],
                                         kpair[0:64, t * NP:(t + 1) * NP],
                                         qT_sb[0:64, p, :],
                                         start=True, stop=True,
                                         tile_position=(0, 0))
                        nc.tensor.matmul(sco[:, SQ:2 * SQ],
                                         kpair[64:128, t * NP:(t + 1) * NP],
                                         qT_sb[64:128, p, :],
                                         start=True, stop=True,
                                         tile_position=(64, 0))
                        att_t = attp.tile([NP, 2 * SQ], BF16, tag="at", name="att_t")
                        nc.scalar.activation(out=att_t, in_=sco, func=AF.Gelu,
                                             scale=SCALE)
                        nc.tensor.matmul(ctx_ps[0:64, :], vq[:, t, c0:c0 + 64],
                                         att_t[:, 0:SQ],
                                         start=(t == 0), stop=(t == NT - 1),
                                         tile_position=(0, 0))
                        nc.tensor.matmul(ctx_ps[64:128, :], vq[:, t, c0 + 64:c0 + NP],
                                         att_t[:, SQ:2 * SQ],
                                         start=(t == 0), stop=(t == NT - 1),
                                         tile_position=(0, 64))
                    nc.vector.tensor_copy(ctxT_sb[:, p, :], ctx_ps)

                prepare(0)
                v_half(0)
                prepare(1)
                attn(0)
                v_half(1)
                prepare(2)
                attn(1)
                prepare(3)
                attn(2)
                prepare(4)
                attn(3)
                prepare(5)
                attn(4)
                prepare(6)
                attn(5)
                prepare(7)
                attn(6)
                attn(7)

            # ------------- out proj, gate + epilogue -----------------------
            with tc.tile_pool(name="w2", bufs=2) as w2, \
                 tc.tile_pool(name="big", bufs=1) as big, \
                 tc.tile_pool(name="pp2", bufs=4, space="PSUM") as pp2:
                wo_sb = w2.tile([NP, KC, DM], F32R, tag="w2t", name="wo_sb")
                for dt in range(KC):
                    nc.sync.dma_start(out=wo_sb[:, :, dt * NP:(dt + 1) * NP],
                                      in_=wo_v[:, :, dt * NP:(dt + 1) * NP])
                bo_bc = big.tile([NP, DM], F32)
                nc.sync.dma_start(out=bo_bc, in_=_bcast(bo_d))
                outT_sb = big.tile([NP, KC, SQ], F32R)
                for dt in range(KC):
                    ps_t = pp2.tile([NP, SQ], F32, tag="po", name="ps_t")
                    for dc in range(KC):
                        nc.tensor.matmul(
                            ps_t,
                            wo_sb[:, dc, dt * NP:(dt + 1) * NP],
                            ctxT_sb[:, dc, :],
                            start=(dc == 0), stop=(dc == KC - 1))
                    nc.vector.tensor_scalar_add(outT_sb[:, dt, :], ps_t,
                                                bo_sb[:, dt:dt + 1])
                orow_sb = big.tile([NP, NST, DM], F32)
                for st in range(NST):
                    for ns in range(DM // N512):
                        ps_t = pp2.tile([NP, N512], F32, tag="po", name="ps_t")
                        for dc in range(KC):
                            nc.tensor.matmul(
                                ps_t,
                                ctxT_sb[:, dc, st * NP:(st + 1) * NP],
                                wo_sb[:, dc, ns * N512:(ns + 1) * N512],
                                start=(dc == 0), stop=(dc == KC - 1))
                        nc.vector.tensor_add(
                            orow_sb[:, st, ns * N512:(ns + 1) * N512],
                            ps_t, bo_bc[:, ns * N512:(ns + 1) * N512])
                wg_sb = w2.tile([NP, KC, DM], F32R, tag="w2t", name="wg_sb")
                for dt in range(KC):
                    nc.sync.dma_start(out=wg_sb[:, :, dt * NP:(dt + 1) * NP],
                                      in_=wg_v[:, :, dt * NP:(dt + 1) * NP])
                bg_bc = big.tile([NP, DM], F32)
                gam_bc = big.tile([NP, DM], F32)
                bet_bc = big.tile([NP, DM], F32)
                nc.sync.dma_start(out=bg_bc, in_=_bcast(bg_d))
                nc.sync.dma_start(out=gam_bc, in_=_bcast(gam_d))
                nc.sync.dma_start(out=bet_bc, in_=_bcast(bet_d))
                xr_sb = big.tile([NP, NST, DM], F32)
                nc.sync.dma_start(out=xr_sb,
                                  in_=xr_d.rearrange("(n p) d -> p n d", p=NP))

                gate_sb = big.tile([NP, NST, DM], F32)
                t1_sb = big.tile([NP, NST, DM], F32)
                stats = pers.tile([NP, 2, 6], F32)
                mv = pers.tile([NP, 2], F32)
                std = pers.tile([NP, 1], F32)
                rstd = pers.tile([NP, 1], F32)
                y_sb = gate_sb
                for st in range(NST):
                    for ns in range(DM // N512):
                        ps_t = pp2.tile([NP, N512], F32, tag="po", name="ps_t")
                        for dc in range(KC):
                            nc.tensor.matmul(
                                ps_t,
                                outT_sb[:, dc, st * NP:(st + 1) * NP],
                                wg_sb[:, dc, ns * N512:(ns + 1) * N512],
                                start=(dc == 0), stop=(dc == KC - 1))
                        nc.vector.tensor_add(
                            gate_sb[:, st, ns * N512:(ns + 1) * N512],
                            ps_t, bg_bc[:, ns * N512:(ns + 1) * N512])
                    nc.scalar.activation(out=gate_sb[:, st, :],
                                         in_=gate_sb[:, st, :], func=AF.Sigmoid)
                    # y_pre = gate*(out - x) + 2x
                    nc.vector.tensor_sub(t1_sb[:, st, :], orow_sb[:, st, :],
                                         xr_sb[:, st, :])
                    nc.vector.tensor_mul(orow_sb[:, st, :], t1_sb[:, st, :],
                                         gate_sb[:, st, :])
                    nc.vector.scalar_tensor_tensor(
                        out=t1_sb[:, st, :], in0=xr_sb[:, st, :], scalar=2.0,
                        in1=orow_sb[:, st, :], op0=ALU.mult, op1=ALU.add)
                    # layernorm over DM
                    yv = t1_sb[:, st, :].rearrange("p (g d) -> p g d", g=2)
                    for g in range(2):
                        nc.vector.bn_stats(out=stats[:, g, :], in_=yv[:, g, :])
                    nc.vector.bn_aggr(out=mv, in_=stats)
                    nc.scalar.activation(out=std, in_=mv[:, 1:2], func=AF.Sqrt,
                                         bias=eps_sb)
                    nc.vector.reciprocal(rstd, std)
                    nc.vector.tensor_scalar(
                        out=orow_sb[:, st, :], in0=t1_sb[:, st, :],
                        scalar1=mv[:, 0:1], scalar2=rstd,
                        op0=ALU.subtract, op1=ALU.mult)
                    nc.vector.tensor_mul(orow_sb[:, st, :], orow_sb[:, st, :],
                                         gam_bc)
                    nc.vector.tensor_add(y_sb[:, st, :], orow_sb[:, st, :],
                                         bet_bc)
                    nc.sync.dma_start(
                        out=y_d.rearrange("(n p) d -> p n d", p=NP)[:, st, :],
                        in_=y_sb[:, st, :])

    nc.compile()
    return nc


def kernel(x, Wq, bq, Wk, bk, Wv, bv, Wo, bo, Wg, bg, attention_weights,
           ln_gamma, ln_beta):
    x = np.asarray(x, dtype=np.float32)
    f32 = lambda a: np.ascontiguousarray(np.asarray(a, dtype=np.float32))
    Wq, Wk, Wv, Wo, Wg = map(f32, (Wq, Wk, Wv, Wo, Wg))
    bq, bk, bv, bo, bg = map(f32, (bq, bk, bv, bo, bg))
    aw, gam, bet = map(f32, (attention_weights, ln_gamma, ln_beta))

    if "nc" not in _CACHE:
        _CACHE["nc"] = _build()
    nc = _CACHE["nc"]

    # fold softmax(attention_weights) into Wv / bv
    e = np.exp(aw - aw.max())
    head_w = (e / e.sum()).astype(np.float32)
    hw_exp = np.repeat(head_w, HD)              # [DM]
    Wv_s = Wv * hw_exp[:, None]
    bv_s = bv * hw_exp

    wqT = np.ascontiguousarray(Wq.T)
    wkT = np.ascontiguousarray(Wk.T)
    wvT = np.ascontiguousarray(Wv_s.T)
    woT = np.ascontiguousarray(Wo.T)
    wgT = np.ascontiguousarray(Wg.T)

    in_maps = []
    for c in range(8):
        b, blk = divmod(c, 4)
        r0 = blk * SQ
        xb = x[b]
        perm = np.r_[r0:r0 + SQ, 0:r0, r0 + SQ:S]
        in_maps.append({
            "xT": np.ascontiguousarray(xb[perm].T),
            "xr": np.ascontiguousarray(xb[r0:r0 + SQ]),
            "wkT": wkT, "wvT": wvT, "wqT": wqT, "woT": woT, "wgT": wgT,
            "bq": bq, "bk": bk, "bv": bv_s, "bo": bo, "bg": bg,
            "gam": gam, "bet": bet,
        })

    res = run_bass_kernel_spmd(nc, in_maps, core_ids=list(range(8)),
                               trace=_TRACE[0])
    _LAST_RESULT[0] = res

    y = np.empty((B, S, DM), dtype=np.float32)
    for c in range(8):
        b, blk = divmod(c, 4)
        r0 = blk * SQ
        y[b, r0:r0 + SQ] = res.results[c]["y"]
    return y
